# revision 11
# baseline (speedup 1.0000x reference)
"""Trainium2 Bass kernel for nn_Convolution_58171037057365.

out = skip_linear(x) + conv3d(x, K(tp_weight)); the skip folds into the conv
kernel's center tap, so the device work is one 'same'-padded 5x5x5 conv over
[2,64,48,48,48].

The metric here is end-to-end wall clock of kernel(), and the axon tunnel to
the 8 NeuronCores moves ~35-40 MB/s half-duplex, so the design minimizes wire
bytes:
  - input x is quantized host-side to int8 (global scale absmax/126, RNE);
    the dequant scale is folded into the conv weights. 18.9 MB up.
  - weights ship in bf16 (1.02 MB per core).
  - output is quantized on device to int8 with a per-chunk per-partition
    absmax scale (60 chunks per core); host dequantizes. 14.3 MB down.
  - donated output buffers are created on device (never cross the wire);
    the jitted executable is built once and cached across calls.
End-to-end quantization error measured ~1.25e-2 max-rel (budget 2e-2).

Distribution: 8 cores = 2 batches x 4 x-slabs (12 output planes each, halo 2).
Each core receives its 16 raw x-planes as int8 [64, 16*48*48]; zero padding
and the z-shift duplicate (for paired-tap K=128 matmuls) are built on device:
SBUF xbuf [128, 16, 52, 53] bf16, partitions 64-127 hold the z+1-shifted copy.

Device per chunk (one x-plane, <=10 y-rows, 48 z): 50 paired K=128 matmuls +
25 single K=64 matmuls accumulate in PSUM; DVE computes per-partition absmax,
rescales to +-126 with RNE (magic-constant rounding), writes int8.
"""

import numpy as np
import ml_dtypes

import jax
import jax.numpy as jnp
from jax.sharding import Mesh, NamedSharding, PartitionSpec
from jax.experimental.shard_map import shard_map

import concourse.bass as bass
import concourse.mybir as mybir
import concourse.tile as tile
from concourse.bass2jax import (_bass_exec_p, install_neuronx_cc_hook,
                                partition_id_tensor)
from concourse.tile_rust import add_dep_helper

# ---- problem geometry (hardcoded) ----
MUL = 16
KS = 5
PAD = 2
R_BASIS = 5
B = 2
C = 64
G = 48                 # grid size
OXN = 12               # output x-planes per core
XL = OXN + 2 * PAD     # local x planes incl halo = 16
YP = G + 2 * PAD       # 52
ZPP = G + 2 * PAD + 1  # 53 (one spare z column for the +1-shifted hi copy)
PL = G * G             # raw plane elements = 2304
XIN = XL * PL          # per-core int8 input elements = 36864
OSP = OXN * G * G      # per-core output spatial = 27648
NCHUNK = OXN * 5       # quantization chunks per core = 60
QMAX = 126.0           # int8 quant ceiling (margin below 127 vs. saturation)
MAGIC = 12582912.0     # 1.5 * 2^23: float32 RNE round-to-integer constant

PW0 = float(np.sqrt(1.0 / (2 * MUL)))
PW1 = float(np.sqrt(3.0 / (2 * MUL)))
INV_SQRT3 = float(1.0 / np.sqrt(3.0))

DT = mybir.dt.bfloat16
DT_NP = ml_dtypes.bfloat16

# chunking of one x-plane's output: (oy0, count)
Y_CHUNKS = ((0, 10), (10, 10), (20, 10), (30, 10), (40, 8))
CHUNK_WIDTHS = [cnt * G for _ in range(OXN) for (_, cnt) in Y_CHUNKS]

# tap order used by both weight packing and the device loop
PAIR_TAPS = [(dx, dy, za) for dx in range(KS) for dy in range(KS) for za in (1, 3)]
SING_TAPS = [(dx, dy) for dx in range(KS) for dy in range(KS)]


def _build_kern(tp_weight, w_sc0, w_sc1):
    """[64(out), 64(in), 5,5,5] conv kernel with the skip linear folded in."""
    r = 2.5
    ax = np.arange(-PAD, PAD + 1.0)
    lattice = np.stack(np.meshgrid(ax, ax, ax, indexing='ij'), axis=-1)
    d = np.linalg.norm(lattice, axis=-1)
    values = np.linspace(0.0, r, R_BASIS + 2)[1:-1]
    step = values[1] - values[0]
    diff = (d[..., None] - values) / step

    def sus(t):
        return np.where(t > 0, np.exp(-1.0 / np.where(t > 0, t, 1.0)), 0.0)

    emb = 1.14136 * np.exp(2.0) * sus(diff + 1.0) * sus(1.0 - diff)
    safe = np.where(d > 0, d, 1.0)
    unit = lattice / safe[..., None]
    Y0 = np.full(d.shape, 1.0 / (2.0 * np.sqrt(np.pi)))
    Y1 = np.sqrt(3.0 / (4.0 * np.pi)) * unit

    W = (emb.reshape(-1, R_BASIS) @ tp_weight).reshape(KS, KS, KS, 4, MUL, MUL) / KS ** 3
    W00, W01, W10, W11 = W[..., 0, :, :], W[..., 1, :, :], W[..., 2, :, :], W[..., 3, :, :]
    Kss = PW0 * Y0[..., None, None] * W00
    Ksv = (PW1 * INV_SQRT3) * np.einsum('xyzuw,xyzm->xyzuwm', W01, Y1)
    Ksv = Ksv.reshape(KS, KS, KS, MUL, 3 * MUL)
    Kvs = (PW0 * INV_SQRT3) * np.einsum('xyzuw,xyzm->xyzumw', W11, Y1)
    Kvs = Kvs.reshape(KS, KS, KS, 3 * MUL, MUL)
    Kvv = (PW1 * INV_SQRT3) * np.einsum('xyzuw,xyz,mn->xyzumwn', W10, Y0, np.eye(3))
    Kvv = Kvv.reshape(KS, KS, KS, 3 * MUL, 3 * MUL)
    top = np.concatenate([Kss, Ksv], axis=-1)
    bot = np.concatenate([Kvs, Kvv], axis=-1)
    M = np.concatenate([top, bot], axis=-2)          # [5,5,5, in, out]
    kern = np.transpose(M, (4, 3, 0, 1, 2)).copy()   # [out, in, kx,ky,kz]

    inv = 1.0 / np.sqrt(MUL)
    S = np.zeros((C, C))
    S[:MUL, :MUL] = w_sc0.T * inv
    vec = MUL + 3 * np.arange(MUL)
    for m in range(3):
        S[np.ix_(vec + m, vec + m)] = w_sc1.T * inv
    kern[:, :, PAD, PAD, PAD] += S
    return kern


def _pack_weights(kern, x_scale):
    """wpair [128, 50*64], wsingle [64, 25*64] in tap order, x dequant scale
    folded in, cast to bf16."""
    kern = kern * x_scale
    wpair = np.zeros((128, len(PAIR_TAPS) * C), np.float64)
    for m, (dx, dy, za) in enumerate(PAIR_TAPS):
        wpair[:C, m * C:(m + 1) * C] = kern[:, :, dx, dy, za].T       # lo: tap dz=za
        wpair[C:, m * C:(m + 1) * C] = kern[:, :, dx, dy, za - 1].T   # hi: tap dz=za-1
    wsingle = np.zeros((C, len(SING_TAPS) * C), np.float64)
    for s, (dx, dy) in enumerate(SING_TAPS):
        wsingle[:, s * C:(s + 1) * C] = kern[:, :, dx, dy, 4].T
    return wpair.astype(DT_NP), wsingle.astype(DT_NP)


def _build_bass():
    nc = bass.Bass("TRN2", target_bir_lowering=False, debug=False, num_devices=8)
    xq = nc.dram_tensor("xq", [C, XIN], mybir.dt.int8, kind="ExternalInput")
    wp = nc.dram_tensor("wpair", [128, len(PAIR_TAPS) * C], DT, kind="ExternalInput")
    ws = nc.dram_tensor("wsingle", [C, len(SING_TAPS) * C], DT, kind="ExternalInput")
    outq = nc.dram_tensor("outq", [C, OSP], mybir.dt.int8, kind="ExternalOutput")
    oscale = nc.dram_tensor("oscale", [C, NCHUNK], mybir.dt.float32,
                            kind="ExternalOutput")

    with tile.TileContext(nc) as tc:
        with (
            tc.tile_pool(name="xpool", bufs=1) as xpool,
            tc.tile_pool(name="wpool", bufs=1) as wpool,
            tc.tile_pool(name="opool", bufs=1) as opool,
            tc.tile_pool(name="qpool", bufs=2) as qpool,
            tc.tile_pool(name="pspool", bufs=2, space="PSUM") as pspool,
        ):
            xstage = xpool.tile([128, XL, G, G], mybir.dt.int8)
            xbuf = xpool.tile([128, XL, YP, ZPP], DT)
            obuf = opool.tile([C, OSP], mybir.dt.int8)
            osc_sb = opool.tile([C, NCHUNK], mybir.dt.float32)
            wp_t = wpool.tile([128, len(PAIR_TAPS) * C], DT)
            ws_t = wpool.tile([C, len(SING_TAPS) * C], DT)

            # Per-plane DMAs paired 1:1 with per-plane DVE cast copies keep
            # every instruction at <=1 semaphore wait (a single big DMA gets
            # striped over many queues and blows the ISA sync-wait budget).
            in_dmas = [nc.sync.dma_start(wp_t[:], wp[:]),
                       nc.sync.dma_start(ws_t[:], ws[:])]
            for a in range(XL):
                seg = xq[:, a * PL:(a + 1) * PL]
                in_dmas.append(nc.sync.dma_start(xstage[0:C, a, :, :], seg))
                in_dmas.append(nc.sync.dma_start(xstage[C:128, a, :, :], seg))

            # device-side zero padding + int8->bf16 dequant-to-integer cast.
            # hi half (partitions 64-127) is the z+1-shifted copy so one K=128
            # matmul contracts two z-adjacent taps. Only the border strips are
            # memset (a whole-xbuf memset would add a WAW wait to every cast
            # copy, blowing the 1-wait ISA budget on DVE TensorCopy).
            for a in range(XL):
                nc.vector.memset(xbuf[:, a, 0:PAD, :], 0.0)
                nc.vector.memset(xbuf[:, a, PAD + G:YP, :], 0.0)
                nc.vector.memset(xbuf[0:C, a, PAD:PAD + G, 0:PAD], 0.0)
                nc.vector.memset(xbuf[0:C, a, PAD:PAD + G, PAD + G:ZPP], 0.0)
                nc.vector.memset(xbuf[C:128, a, PAD:PAD + G, 0:PAD + 1], 0.0)
                nc.vector.memset(xbuf[C:128, a, PAD:PAD + G, PAD + G + 1:ZPP], 0.0)
            for a in range(XL):
                nc.vector.tensor_copy(xbuf[0:C, a, PAD:PAD + G, PAD:PAD + G],
                                      xstage[0:C, a, :, :])
                nc.vector.tensor_copy(xbuf[C:128, a, PAD:PAD + G, PAD + 1:PAD + G + 1],
                                      xstage[C:128, a, :, :])

            # PE observers: absorb the weight-DMA completion waits into PE
            # program order so the first matmul carries only the DVE wait.
            pe_obs = [nc.tensor.ldweights(wp_t[:, 0:2]),
                      nc.tensor.ldweights(ws_t[:, 0:2])]
            first_mm = None

            for ox in range(OXN):
                for yi, (oy0, cnt) in enumerate(Y_CHUNKS):
                    ci = ox * len(Y_CHUNKS) + yi
                    N = cnt * G
                    ps = pspool.tile([C, 480], mybir.dt.float32, tag="ps")
                    n_mm = len(PAIR_TAPS) + len(SING_TAPS)
                    k = 0
                    for m, (dx, dy, za) in enumerate(PAIR_TAPS):
                        rhs = xbuf[:, ox + dx, oy0 + dy:oy0 + dy + cnt, za:za + G]
                        mm = nc.tensor.matmul(ps[:, :N], wp_t[:, m * C:(m + 1) * C], rhs,
                                              start=(k == 0), stop=(k == n_mm - 1))
                        if first_mm is None:
                            first_mm = mm
                            for obs in pe_obs:
                                add_dep_helper(mm.ins, obs.ins, False, "order after lane observers")
                        k += 1
                    for s, (dx, dy) in enumerate(SING_TAPS):
                        rhs = xbuf[0:C, ox + dx, oy0 + dy:oy0 + dy + cnt, 4:4 + G]
                        mm = nc.tensor.matmul(ps[:, :N], ws_t[:, s * C:(s + 1) * C], rhs,
                                              start=(k == 0), stop=(k == n_mm - 1))
                        k += 1

                    # int8 quantization: per-partition chunk absmax am ->
                    # oscale = am/126 (the host dequant scale) -> rec =
                    # 1/oscale -> RNE round via +-MAGIC -> int8 SBUF.
                    ob = ox * G * G + oy0 * G
                    am = qpool.tile([C, 1], mybir.dt.float32, tag="am")
                    rec = qpool.tile([C, 1], mybir.dt.float32, tag="rec")
                    tmpf = qpool.tile([C, 480], mybir.dt.float32, tag="tmpf")
                    nc.vector.tensor_reduce(am[:], ps[:, :N],
                                            axis=mybir.AxisListType.X,
                                            op=mybir.AluOpType.max,
                                            apply_absolute_value=True)
                    nc.vector.tensor_scalar_mul(osc_sb[:, ci:ci + 1], am[:],
                                                1.0 / QMAX)
                    nc.vector.reciprocal(rec[:], osc_sb[:, ci:ci + 1])
                    nc.vector.tensor_scalar(tmpf[:, :N], ps[:, :N], rec[:], MAGIC,
                                            mybir.AluOpType.mult,
                                            mybir.AluOpType.add)
                    last_q = nc.vector.tensor_scalar(obuf[:, ob:ob + N], tmpf[:, :N],
                                                     MAGIC, None,
                                                     mybir.AluOpType.subtract)

            # Output DMAs on the ACT HWDGE ring. Before each, a tiny ACT read
            # of the segment's last-written element absorbs the DVE data-ready
            # wait into ACT program order.
            SEG = 4 * G * G
            out_dmas = []
            scr_cps = []
            prev = None
            for j in range(3):
                scr = opool.tile([C, 1], mybir.dt.float32, tag="scr", bufs=4)
                scr_cps.append(nc.scalar.copy(
                    scr[:, :], obuf[:, (j + 1) * SEG - 1:(j + 1) * SEG]))
                out_dmas.append(
                    nc.scalar.dma_start(outq[:, j * SEG:(j + 1) * SEG],
                                        obuf[:, j * SEG:(j + 1) * SEG]))
            scr = opool.tile([C, 1], mybir.dt.float32, tag="scr", bufs=4)
            scr_cps.append(nc.scalar.copy(scr[:, :], osc_sb[:, NCHUNK - 1:NCHUNK]))
            out_dmas.append(nc.scalar.dma_start(oscale[:], osc_sb[:]))
            # hard-order the ACT output section (scr0,dma0,...,scr3,dma3) so
            # the scheduler cannot hoist a DMA above the scr copy that absorbs
            # its DVE data wait.
            chain = [v for pair in zip(scr_cps, out_dmas) for v in pair]
            for a2, b2 in zip(chain[1:], chain[:-1]):
                add_dep_helper(a2.ins, b2.ins, False, "order ACT output section")

            # Pre-absorb all completions into SP program order with a chain of
            # single-wait NOPs; the tail drain's redundant waits are then
            # stripped post-trace.
            for d in in_dmas + out_dmas + scr_cps + [mm, last_q]:
                nop = nc.sync.nop()
                add_dep_helper(nop.ins, d.ins, True, "tail fan-in")
    _strip_tail_drain_waits(nc)
    return nc


def _strip_tail_drain_waits(nc):
    """Remove semaphore waits from the SP tail Drain that are already covered
    by the preceding single-wait NOP chain on the same engine (SP program
    order makes them redundant)."""
    covered = {}  # sem id -> max waited value by earlier SP insts
    for fn in nc.m.functions:
        for bb in fn.blocks:
            for inst in bb.instructions:
                if str(inst.engine) != 'EngineType.SP':
                    continue
                si = inst.sync_info
                if si is None:
                    continue
                if type(inst).__name__ == 'InstDrain' and si.on_wait:
                    kept = [w for w in si.on_wait
                            if covered.get(w.id, -1) < w.wait_value]
                    if len(kept) < len(si.on_wait):
                        si.on_wait = kept
                for w in (si.on_wait or []):
                    if w.wait_value is not None:
                        covered[w.id] = max(covered.get(w.id, -1), w.wait_value)


class _Results:
    """test.py compatibility stub (profiling unavailable under this axon)."""
    exec_time_ns = None
    mean_exec_time_ns = None
    max_exec_time_core_id = None
    instructions_and_trace = None
    profile_json = None
    results = None


_STATE = {}
_RUN_KWARGS = {}   # kept for test.py compatibility; unused
LAST_RESULTS = _Results()


def _ensure_built():
    if 'sharded' in _STATE:
        return
    install_neuronx_cc_hook()
    nc = _build_bass()

    partition_name = nc.partition_id_tensor.name if nc.partition_id_tensor else None
    in_names, out_names, out_avals = [], [], []
    for alloc in nc.m.functions[0].allocations:
        if not isinstance(alloc, mybir.MemoryLocationSet):
            continue
        name = alloc.memorylocations[0].name
        if alloc.kind == "ExternalInput":
            if name != partition_name:
                in_names.append(name)
        elif alloc.kind == "ExternalOutput":
            out_names.append(name)
            out_avals.append(jax.core.ShapedArray(
                tuple(alloc.tensor_shape), mybir.dt.np(alloc.dtype)))
    n_params = len(in_names)
    n_outs = len(out_avals)
    all_names = list(in_names) + list(out_names)
    if partition_name is not None:
        all_names.append(partition_name)
    donate = tuple(range(n_params, n_params + n_outs))

    def _body(*args):
        operands = list(args)
        if partition_name is not None:
            operands.append(partition_id_tensor())
        outs = _bass_exec_p.bind(
            *operands,
            out_avals=tuple(out_avals),
            in_names=tuple(all_names),
            out_names=tuple(out_names),
            lowering_input_output_aliases=(),
            sim_require_finite=True,
            sim_require_nnan=True,
            nc=nc,
        )
        return tuple(outs)

    devices = jax.devices()[:8]
    mesh = Mesh(np.asarray(devices), ("core",))
    spec = NamedSharding(mesh, PartitionSpec("core"))
    in_specs = (PartitionSpec("core"),) * (n_params + n_outs)
    out_specs = (PartitionSpec("core"),) * n_outs
    sharded = jax.jit(
        shard_map(_body, mesh=mesh, in_specs=in_specs, out_specs=out_specs,
                  check_rep=False),
        donate_argnums=donate, keep_unused=True)

    def make_zeros():
        return (jnp.zeros((8 * C, OSP), jnp.int8),
                jnp.zeros((8 * C, NCHUNK), jnp.float32))
    make_zeros = jax.jit(make_zeros, out_shardings=(spec, spec))

    _STATE.update(dict(sharded=sharded, make_zeros=make_zeros, devices=devices,
                       spec=spec, mesh=mesh,
                       widths=np.asarray(CHUNK_WIDTHS, np.int64)))


def kernel(x, w_sc0, w_sc1, tp_weight):
    x = np.asarray(x)
    _ensure_built()
    devices, spec = _STATE['devices'], _STATE['spec']

    absmax = float(np.abs(x).max())
    s = QMAX / absmax if absmax > 0 else 0.0
    x_scale = absmax / QMAX if absmax > 0 else 1.0

    kern = _build_kern(np.asarray(tp_weight, np.float64),
                       np.asarray(w_sc0, np.float64),
                       np.asarray(w_sc1, np.float64))
    wpair, wsingle = _pack_weights(kern, x_scale)

    zeros = _STATE.pop('zeros_next', None)
    if zeros is None:
        zeros = _STATE['make_zeros']()

    # quantize + upload per core, pipelined: device_put is async, so the wire
    # streams core c while core c+1 is being quantized on host.
    xq_parts = []
    for core in range(8):
        b, sx0 = core // 4, (core % 4) * OXN
        g0, g1 = sx0 - PAD, sx0 + OXN + PAD
        c0, c1 = max(g0, 0), min(g1, G)
        slab = np.zeros((C, XL, PL), np.int8)
        blk = x[b, :, c0:c1].reshape(C, c1 - c0, PL)
        slab[:, c0 - g0:c1 - g0, :] = np.rint(blk * s).astype(np.int8)
        xq_parts.append(jax.device_put(slab.reshape(C, XIN), devices[core]))
    wp_parts = [jax.device_put(wpair, d) for d in devices]
    ws_parts = [jax.device_put(wsingle, d) for d in devices]

    xq_g = jax.make_array_from_single_device_arrays((8 * C, XIN), spec, xq_parts)
    wp_g = jax.make_array_from_single_device_arrays((8 * 128, wpair.shape[1]), spec, wp_parts)
    ws_g = jax.make_array_from_single_device_arrays((8 * C, wsingle.shape[1]), spec, ws_parts)

    outq_g, osc_g = _STATE['sharded'](xq_g, wp_g, ws_g, *zeros)
    # prebuild next call's donated zero buffers on device, async
    _STATE['zeros_next'] = _STATE['make_zeros']()

    outq = np.asarray(outq_g)                       # [512, 27648] int8
    osc = np.asarray(osc_g)                         # [512, 60] f32 (chunk absmax)

    sc_full = np.repeat(osc, _STATE['widths'], axis=1)
    y_flat = outq.astype(np.float32) * sc_full
    y = np.empty((B, C, G, G, G), np.float32)
    for core in range(8):
        b, sx0 = core // 4, (core % 4) * OXN
        y[b, :, sx0:sx0 + OXN] = y_flat[core * C:(core + 1) * C].reshape(C, OXN, G, G)
    return y
